# revision 38
# baseline (speedup 1.0000x reference)
"""Trainium2 Bass kernel for the DiseaseGNN problem (2x SAGEConv + edge MLP).

Strategy (8 NeuronCores, SPMD):
  - Edges sorted by dst; core k owns dst range [12500k, 12500(k+1)).
  - Aggregation = one-hot matmuls accumulated in PSUM per 128-node window
    (no scatter needed). Counts ride along as a ones-column in the gathered
    message tile.
  - Per-edge gathers via the dma_gather extended instruction (int16 indices);
    tables are laid out as 8 blocks of 12544 padded rows (100352 total) and
    split into 4 chunks of 25088 rows so local indices fit in int16.
  - h = relu(W_l @ mean + W_r @ h_prev) computed in node space per window.
  - Classifier folded into node space: u = h2@wc1a.T + bc1, v = h2@wc1b.T,
    hidden = relu(u[src] + v[dst]), out = hidden@wc2.T + bc2, with u and v
    packed as one [u|v] per-node table: the AllGathered copy serves u[src],
    the local copy serves v[dst] (dst is always core-local).
  - x is shipped as one 1.6MB block per core and AllGathered + expanded to
    the 256-byte-row gather table on device; its transpose (for the lin_r
    term) is built on device with a DMA transpose.
  - Host<->device traffic minimized: indices shipped un-replicated (the
    2x16-row band replication the gather needs is done on device), all
    weights/biases packed into two params, iota packed into the dst param,
    fp16 output.
"""
import os
import sys
import numpy as np

for _p in ('/opt/trn_rl_repo',):
    if _p not in sys.path:
        sys.path.insert(0, _p)

# Persistent compilation caches: run_bass_kernel_spmd re-jits its closure on
# every call, so without a persistent cache each call pays the full XLA
# compile (~4s). With it, repeat calls hit disk.
try:
    import jax
    _cache_dir = os.path.expanduser("~/.cache/bass_jax_cache")
    os.makedirs(_cache_dir, exist_ok=True)
    jax.config.update("jax_compilation_cache_dir", _cache_dir)
    jax.config.update("jax_persistent_cache_min_entry_size_bytes", 0)
    jax.config.update("jax_persistent_cache_min_compile_time_secs", 0.0)
except Exception:
    pass

import concourse.bass as bass
import concourse.bacc as bacc
import concourse.mybir as mybir
import concourse.tile as tile
from concourse.bass import ds
from concourse.bass_utils import run_bass_kernel_spmd

f32 = mybir.dt.float32
fp16 = mybir.dt.float16
i16 = mybir.dt.int16
i8 = mybir.dt.int8

N = 100000
E = 1600000
NCORES = 8
NS = N // NCORES            # 12500 nodes per core
W = 128                      # node window
NWIN = (NS + W - 1) // W     # 98 windows per core (last partial)
BLK = NWIN * W               # 12544 padded table rows per core block
TROWS = NCORES * BLK         # 100352
NCHUNK = 4
CHUNK = TROWS // NCHUNK      # 25088 (< 32767 so int16 indices work)
GRP = 2                      # windows per gather group
NGRP = NWIN // GRP           # 49

TRACE = False
LAST_EXEC_TIME_NS = None
LAST_RUN_WALL_NS = None
PHASES = 3                   # debug: 1=layer1 only, 2=+layer2, 3=full
Q4 = True                    # debug: rotate gathers across 4 SWDGE queues

_NC_CACHE = {}

RELU = mybir.ActivationFunctionType.Relu
IDENT = mybir.ActivationFunctionType.Identity
EQ = mybir.AluOpType.is_equal
MUL = mybir.AluOpType.mult


def _build(tpc, phases=3, q4=True):
    TPW = NCHUNK * tpc                 # tiles per window
    CALL = GRP * tpc * W               # idx per (group, chunk) gather call
    SLOTS_COLS = NWIN * TPW * 2        # out2 cols
    IDXC = NGRP * (CALL // 16)         # idx columns (i16, wrap-16)

    nc = bacc.Bacc(num_swdge_queues=4)

    # The axon transfer path has ~100ms fixed cost PER BUFFER, so all inputs
    # ship as one packed fp16 blob, unpacked on device via bitcast APs.
    # Section lengths in fp16 halfwords (all even, so f32 views stay aligned):
    WTOT = 128 * 128 + 64 * 128 * 2 + 128 * 64 * 2 + 64 * 64 * 2 + 128 * 128
    L_XC = BLK * 64                    # x block, fp16
    L_SRC = 64 * IDXC                  # src idx bands, i16 (un-replicated)
    L_VDL = 64 * IDXC // 2             # v dst-local bands, i8
    L_DST = 128 * (NWIN * TPW + 128) // 2   # dst one-hot operand + iota, i8
    L_W = WTOT                         # packed fp16 consts: ident w1lT w1rT
    #                                    w2lT w2rT wc1aT wc1bT wc2rep
    L_B = 512 * 2                      # packed f32 biases: b1l b2l bc1 bc2rep
    O_XC = 0
    O_SRC = O_XC + L_XC
    O_VDL = O_SRC + L_SRC
    O_DST = O_VDL + L_VDL
    O_W = O_DST + L_DST
    O_B = O_W + L_W
    TOTH = O_B + L_B
    blob = nc.declare_dram_parameter("blob", [TOTH], fp16, isOutput=False)
    # pos-major int8 output; per-group col blocks are [ch, win-in-grp,
    # chunk, tile], dynamically quantized per (partition, group) with the
    # scales shipped in qs
    out2 = nc.declare_dram_parameter("out2", [128, SLOTS_COLS], i8, isOutput=True)
    qs = nc.declare_dram_parameter("qs", [128, NGRP], f32, isOutput=True)
    if phases < 3:
        out_h1 = nc.declare_dram_parameter("out_h1", [128, BLK], fp16, isOutput=True)
        out_u = nc.declare_dram_parameter("out_u", [BLK, 128], fp16, isOutput=True)
        out_v = nc.declare_dram_parameter("out_v", [BLK, 128], fp16, isOutput=True)

    xc_b = nc.dram_tensor("xc_bounce", [BLK, 64], fp16)
    xc_full = nc.dram_tensor("xc_full", [TROWS, 64], fp16, addr_space="Shared")
    xp = nc.dram_tensor("xp_int", [TROWS, 128], fp16)
    h1_local = nc.dram_tensor("h1_local", [BLK, 128], fp16)
    h1_full = nc.dram_tensor("h1_full", [TROWS, 128], fp16, addr_space="Shared")
    u_local = nc.dram_tensor("u_local", [BLK, 128], fp16)
    u_full = nc.dram_tensor("u_full", [TROWS, 128], fp16, addr_space="Shared")

    with tile.TileContext(nc) as tc:
        with (
            tc.tile_pool(name="const", bufs=1) as const,
            tc.tile_pool(name="resident", bufs=1) as res,
        ):
            # --- unpack consts ---
            woff = [O_W]

            def wload(p, cols):
                t = const.tile([p, cols], fp16, tag=f"w{woff[0]}")
                nc.sync.dma_start(
                    out=t[:],
                    in_=blob[woff[0]:woff[0] + p * cols].rearrange(
                        "(p x) -> p x", p=p))
                woff[0] += p * cols
                return t

            ident_sb = wload(128, 128)
            w1lT_sb = wload(64, 128)
            w1rT_sb = wload(64, 128)
            w2lT_sb = wload(128, 64)
            w2rT_sb = wload(128, 64)
            wc1aT_sb = wload(64, 64)
            wc1bT_sb = wload(64, 64)
            wc2r_sb = wload(128, 128)   # wc2 rows replicated: [p, ch*64+f]

            boff = [O_B]

            def bload(p, cols):
                t = const.tile([p, cols], f32, tag=f"b{boff[0]}")
                nc.sync.dma_start(
                    out=t[:],
                    in_=blob[boff[0]:boff[0] + p * cols * 2].bitcast(
                        f32).rearrange("(p x) -> p x", p=p))
                boff[0] += p * cols * 2
                return t

            b1l_sb = bload(128, 1)
            b2l_sb = bload(64, 1)
            bc1_sb = bload(64, 1)
            bc2r_sb = bload(128, 2)

            # resident idx/dst tables, loaded once and reused by all phases
            src_idx = res.tile([128, IDXC], i16)
            vdl8 = res.tile([128, IDXC], i8)
            v_idx = res.tile([128, IDXC], i16)
            dst_all = res.tile([128, NWIN * TPW + 128], i8)
            nc.sync.dma_start(
                out=dst_all[:],
                in_=blob[O_DST:O_DST + L_DST].bitcast(i8).rearrange(
                    "(p x) -> p x", p=128))
            for c in range(NCHUNK):
                for h in range(2):
                    nc.sync.dma_start(
                        out=src_idx[32 * c + 16 * h:32 * c + 16 * (h + 1), :],
                        in_=blob[O_SRC + 16 * c * IDXC:
                                 O_SRC + 16 * (c + 1) * IDXC].bitcast(
                            i16).rearrange("(r x) -> r x", r=16))
                    nc.sync.dma_start(
                        out=vdl8[32 * c + 16 * h:32 * c + 16 * (h + 1), :],
                        in_=blob[O_VDL + 8 * c * IDXC:
                                 O_VDL + 8 * (c + 1) * IDXC].bitcast(
                            i8).rearrange("(r x) -> r x", r=16))

            # x in SBUF, window-major: xcb_t[p, a, d] = xc[a*128+p, d]
            xcb_t = res.tile([128, NWIN, 64], fp16)
            nc.sync.dma_start(
                out=xcb_t[:],
                in_=blob[O_XC:O_XC + L_XC].rearrange(
                    "(a p d) -> p a d", p=128, d=64))
            # x^T built on device with per-window PE transposes (saves
            # shipping a second copy of x)
            xT_sb = res.tile([64, BLK], fp16)

            h1T_all = res.tile([128, BLK], fp16)
            recip_all = res.tile([128, NWIN], f32)
            qs_all = res.tile([128, NGRP], f32)

            HCALL = tpc * W                      # 640 idx per (window, chunk) call
            HC16 = HCALL // 16

            # v idx (dst row in the local u|v table) = dst_local + 128*w,
            # derived on device: iota makes the per-window 128*w offsets
            wofs = res.tile([128, IDXC], i16)
            nc.gpsimd.iota(wofs[:], pattern=[[128, NWIN], [0, HC16]],
                           channel_multiplier=0)
            nc.vector.tensor_tensor(out=v_idx[:], in0=vdl8[:], in1=wofs[:],
                                    op=mybir.AluOpType.add)

            def gather_win(idx_src, msgp, w, table):
                """Per-(window, chunk) gathers: 640 idx = 41 ring descs, under
                the ~64-desc SWDGE carveout ring limit. Queue c's Q7 pair
                reads idx partitions 32c..32c+31. `w` may be symbolic."""
                msgs = []
                for c in range(NCHUNK):
                    m = msgp.tile([128, tpc, 128], fp16, tag=f"msg{c}")
                    nc.gpsimd.dma_gather(
                        out_ap=m[:], in_ap=table[c * CHUNK:(c + 1) * CHUNK, :],
                        idxs_ap=idx_src[:, ds(w * HC16, HC16)],
                        num_idxs=HCALL, num_idxs_reg=HCALL,
                        elem_size=128, queue_num=c)
                    msgs.append(m)
                return msgs

            def onehot_for(ohp, w):
                oh = ohp.tile([128, TPW, 128], fp16)
                nc.vector.tensor_tensor(
                    out=oh[:],
                    in0=dst_all[:, ds(w * TPW, TPW)].rearrange(
                        "p (b o) -> p b o", o=1).to_broadcast([128, TPW, 128]),
                    in1=dst_all[:, NWIN * TPW:].rearrange(
                        "p (o q) -> p o q", o=1).to_broadcast([128, TPW, 128]),
                    op=EQ)
                return oh

            # AllGather each core's x block, then expand to the 256B-row
            # gather table (cols 0:64 = x, col 64 = 1.0) on device; ships
            # 1.6MB instead of 12.8MB of x per core.
            nc.sync.dma_start(
                out=xc_b[:].rearrange("(a p) d -> p a d", p=128),
                in_=xcb_t[:])
            with (
                tc.tile_pool(name="xw", bufs=3) as xwp,
                tc.tile_pool(name="psXT", bufs=4, space="PSUM") as psxt,
            ):
                with tc.For_i(0, NWIN, 1) as w:
                    xw = xwp.tile([128, 64], fp16, tag="xw")
                    nc.vector.tensor_copy(
                        xw[:],
                        xcb_t[:, ds(w, 1), :].rearrange("p a d -> p (a d)"))
                    xtp = psxt.tile([64, 128], fp16, tag="xt")
                    nc.tensor.transpose(xtp[:], xw[:], ident_sb[:])
                    nc.vector.tensor_copy(xT_sb[:, ds(w * 128, 128)], xtp[:])
            tc.strict_bb_all_engine_barrier()
            with tc.tile_critical():
                with nc.semaphore() as cc_sem:
                    nc.gpsimd.collective_compute(
                        "AllGather", mybir.AluOpType.bypass,
                        ins=[xc_b[:]], outs=[xc_full[:]],
                        replica_groups=[list(range(NCORES))],
                    ).then_inc(cc_sem, 1)
                    nc.gpsimd.wait_ge(cc_sem, 1)
            tc.strict_bb_all_engine_barrier()
            with tc.tile_pool(name="expand", bufs=2) as exp_p:
                ones_t = exp_p.tile([128, NWIN, 1], fp16, tag="ones")
                nc.vector.memset(ones_t[:], 1.0)
                for blk_i in range(NCORES):
                    r0 = blk_i * BLK
                    t = exp_p.tile([128, NWIN, 64], fp16, tag="xstripe")
                    nc.sync.dma_start(
                        out=t[:],
                        in_=xc_full[r0:r0 + BLK, :].rearrange("(a p) d -> p a d", p=128))
                    nc.sync.dma_start(
                        out=xp[r0:r0 + BLK, 0:64].rearrange("(a p) d -> p a d", p=128),
                        in_=t[:])
                    nc.sync.dma_start(
                        out=xp[r0:r0 + BLK, 64:65].rearrange("(a p) d -> p a d", p=128),
                        in_=ones_t[:])
            tc.strict_bb_all_engine_barrier()

            # ---------------- Phase A: layer 1 ----------------
            with (
                tc.tile_pool(name="msgA", bufs=3) as msgp,
                tc.tile_pool(name="ohA", bufs=3) as ohp,
                tc.tile_pool(name="smallA", bufs=4) as smp,
                tc.tile_pool(name="psAggA", bufs=2, space="PSUM") as psagg,
                tc.tile_pool(name="psTrA", bufs=3, space="PSUM") as pstr,
                tc.tile_pool(name="psHA", bufs=2, space="PSUM") as psh,
            ):
                with tc.For_i(0, NWIN, 1) as w:
                    msgs = gather_win(src_idx, msgp, w, xp)
                    oh = onehot_for(ohp, w)
                    agg = psagg.tile([128, 65], f32)
                    for c in range(NCHUNK):
                        for t in range(tpc):
                            nc.tensor.matmul(
                                out=agg[:], lhsT=oh[:, c * tpc + t, :],
                                rhs=msgs[c][:, t, :65],
                                start=(c == 0 and t == 0),
                                stop=(c == NCHUNK - 1 and t == tpc - 1))
                    cntm = smp.tile([128, 1], f32, tag="cnt")
                    nc.vector.tensor_scalar_max(cntm[:], agg[:, 64:65], 1.0)
                    recip = smp.tile([128, 1], f32, tag="recip")
                    nc.vector.reciprocal(recip[:], cntm[:])
                    nc.vector.tensor_copy(recip_all[:, ds(w, 1)], recip[:])
                    mean = smp.tile([128, 64], fp16, tag="mean")
                    nc.vector.tensor_tensor(
                        out=mean[:], in0=agg[:, :64],
                        in1=recip[:].to_broadcast([128, 64]), op=MUL)
                    meanT_ps = pstr.tile([64, 128], fp16, tag="tr")
                    nc.tensor.transpose(meanT_ps[:], mean[:], ident_sb[:])
                    meanT = smp.tile([64, 128], fp16, tag="meanTs")
                    nc.vector.tensor_copy(meanT[:], meanT_ps[:])
                    h1ps = psh.tile([128, 128], f32, tag="h1")
                    nc.tensor.matmul(h1ps[:], lhsT=w1lT_sb[:], rhs=meanT[:], start=True, stop=False)
                    nc.tensor.matmul(h1ps[:], lhsT=w1rT_sb[:], rhs=xT_sb[:, ds(w * 128, 128)],
                                     start=False, stop=True)
                    h1T = smp.tile([128, 128], fp16, tag="h1T")
                    nc.scalar.activation(out=h1T[:], in_=h1ps[:],
                                         func=RELU, bias=b1l_sb[:])
                    nc.vector.tensor_copy(h1T_all[:, ds(w * 128, 128)], h1T[:])
                    h1nm_ps = pstr.tile([128, 128], fp16, tag="tr")
                    nc.tensor.transpose(h1nm_ps[:], h1T[:], ident_sb[:])
                    h1nm = smp.tile([128, 128], fp16, tag="h1nms")
                    nc.vector.tensor_copy(h1nm[:], h1nm_ps[:])
                    nc.sync.dma_start(out=h1_local[ds(w * 128, 128), :], in_=h1nm[:])

            if phases < 3:
                nc.sync.dma_start(out=out_h1[:], in_=h1T_all[:])
            if phases >= 2:
                tc.strict_bb_all_engine_barrier()
                with tc.tile_critical():
                    with nc.semaphore() as cc_sem:
                        nc.gpsimd.collective_compute(
                            "AllGather", mybir.AluOpType.bypass,
                            ins=[h1_local[:]], outs=[h1_full[:]],
                            replica_groups=[list(range(NCORES))],
                        ).then_inc(cc_sem, 1)
                        nc.gpsimd.wait_ge(cc_sem, 1)
                tc.strict_bb_all_engine_barrier()

                # ------------- Phase B: layer 2 + u/v tables -------------
                with (
                    tc.tile_pool(name="msgB", bufs=3) as msgp,
                    tc.tile_pool(name="ohB", bufs=3) as ohp,
                    tc.tile_pool(name="smallB", bufs=4) as smp,
                    tc.tile_pool(name="psAggB", bufs=2, space="PSUM") as psagg,
                    tc.tile_pool(name="psTrB", bufs=3, space="PSUM") as pstr,
                    tc.tile_pool(name="psHB", bufs=3, space="PSUM") as psh,
                ):
                    with tc.For_i(0, NWIN, 1) as w:
                        msgs = gather_win(src_idx, msgp, w, h1_full)
                        oh = onehot_for(ohp, w)
                        agg2 = psagg.tile([128, 128], f32)
                        for c in range(NCHUNK):
                            for t in range(tpc):
                                nc.tensor.matmul(
                                    out=agg2[:], lhsT=oh[:, c * tpc + t, :],
                                    rhs=msgs[c][:, t, :],
                                    start=(c == 0 and t == 0),
                                    stop=(c == NCHUNK - 1 and t == tpc - 1))
                        mean2 = smp.tile([128, 128], fp16, tag="mean2")
                        nc.vector.tensor_tensor(
                            out=mean2[:], in0=agg2[:],
                            in1=recip_all[:, ds(w, 1)].to_broadcast([128, 128]), op=MUL)
                        mean2T_ps = pstr.tile([128, 128], fp16, tag="tr")
                        nc.tensor.transpose(mean2T_ps[:], mean2[:], ident_sb[:])
                        mean2T = smp.tile([128, 128], fp16, tag="m2Ts")
                        nc.vector.tensor_copy(mean2T[:], mean2T_ps[:])
                        h2ps = psh.tile([64, 128], f32, tag="h")
                        nc.tensor.matmul(h2ps[:], lhsT=w2lT_sb[:], rhs=mean2T[:], start=True, stop=False)
                        nc.tensor.matmul(h2ps[:], lhsT=w2rT_sb[:], rhs=h1T_all[:, ds(w * 128, 128)],
                                         start=False, stop=True)
                        h2T = smp.tile([64, 128], fp16, tag="h2T")
                        nc.scalar.activation(out=h2T[:], in_=h2ps[:], func=RELU, bias=b2l_sb[:])
                        ups = psh.tile([64, 128], f32, tag="h")
                        nc.tensor.matmul(ups[:], lhsT=wc1aT_sb[:], rhs=h2T[:], start=True, stop=True)
                        uT = smp.tile([64, 128], fp16, tag="uT")
                        nc.scalar.activation(out=uT[:], in_=ups[:], func=IDENT, bias=bc1_sb[:])
                        vps = psh.tile([64, 128], f32, tag="h")
                        nc.tensor.matmul(vps[:], lhsT=wc1bT_sb[:], rhs=h2T[:], start=True, stop=True)
                        vT = smp.tile([64, 128], fp16, tag="vT")
                        nc.vector.tensor_copy(vT[:], vps[:])
                        unm_ps = pstr.tile([128, 64], fp16, tag="tr")
                        nc.tensor.transpose(unm_ps[:], uT[:], ident_sb[0:64, 0:64])
                        unm = smp.tile([128, 64], fp16, tag="unms")
                        nc.vector.tensor_copy(unm[:], unm_ps[:])
                        nc.sync.dma_start(out=u_local[ds(w * 128, 128), 0:64], in_=unm[:])
                        vnm_ps = pstr.tile([128, 64], fp16, tag="tr")
                        nc.tensor.transpose(vnm_ps[:], vT[:], ident_sb[0:64, 0:64])
                        vnm = smp.tile([128, 64], fp16, tag="vnms")
                        nc.vector.tensor_copy(vnm[:], vnm_ps[:])
                        nc.sync.dma_start(out=u_local[ds(w * 128, 128), 64:128], in_=vnm[:])

            if phases >= 3:
                tc.strict_bb_all_engine_barrier()
                with tc.tile_critical():
                    with nc.semaphore() as cc_sem:
                        nc.gpsimd.collective_compute(
                            "AllGather", mybir.AluOpType.bypass,
                            ins=[u_local[:]], outs=[u_full[:]],
                            replica_groups=[list(range(NCORES))],
                        ).then_inc(cc_sem, 1)
                        nc.gpsimd.wait_ge(cc_sem, 1)
                tc.strict_bb_all_engine_barrier()

                # ------------------ Phase C: classifier ------------------
                # hidden = relu(u[src] + v[dst]); out[e,ch] = hidden.wc2[ch]
                # + bc2[ch], all on DVE/ACT: fused add/relu per chunk-call,
                # then per-channel multiply + innermost-dim reduce.
                ADD = mybir.AluOpType.add
                with (
                    tc.tile_pool(name="gC", bufs=3) as gp,
                    tc.tile_pool(name="hC", bufs=3) as hp,
                    tc.tile_pool(name="accC", bufs=2) as accp,
                    tc.tile_pool(name="stripC", bufs=3) as stp,
                ):
                    with tc.For_i(0, NGRP, 1) as g:
                        acc = accp.tile([128, 2, GRP, NCHUNK, tpc], f32)
                        for wi in range(GRP):
                            for c in range(NCHUNK):
                                ub = gp.tile([128, tpc, 128], fp16, tag=f"ubuf{c}")
                                nc.gpsimd.dma_gather(
                                    out_ap=ub[:], in_ap=u_full[c * CHUNK:(c + 1) * CHUNK, :],
                                    idxs_ap=src_idx[:, ds((g * GRP + wi) * HC16, HC16)],
                                    num_idxs=HCALL, num_idxs_reg=HCALL,
                                    elem_size=128, queue_num=c)
                                vb = gp.tile([128, tpc, 128], fp16, tag=f"vbuf{c}")
                                nc.gpsimd.dma_gather(
                                    out_ap=vb[:], in_ap=u_local[:],
                                    idxs_ap=v_idx[:, ds((g * GRP + wi) * HC16, HC16)],
                                    num_idxs=HCALL, num_idxs_reg=HCALL,
                                    elem_size=128, queue_num=c)
                                hs = hp.tile([128, tpc, 64], fp16, tag="hs")
                                nc.vector.tensor_add(
                                    hs[:], ub[:, :, 0:64], vb[:, :, 64:128])
                                hid = hp.tile([128, tpc, 64], fp16, tag="hid")
                                nc.scalar.activation(out=hid[:], in_=hs[:], func=RELU)
                                for ch in range(2):
                                    prod = hp.tile([128, tpc, 64], fp16, tag="prod")
                                    nc.vector.tensor_tensor(
                                        out=prod[:], in0=hid[:],
                                        in1=wc2r_sb[:, ch * 64:(ch + 1) * 64].rearrange(
                                            "p (o f) -> p o f", o=1).to_broadcast(
                                            [128, tpc, 64]),
                                        op=MUL)
                                    nc.vector.tensor_reduce(
                                        out=acc[:, ch, wi, c, :], in_=prod[:],
                                        axis=mybir.AxisListType.X, op=ADD)
                        biased = stp.tile([128, 2, GRP * TPW], f32, tag="biased")
                        for ch in range(2):
                            nc.vector.tensor_scalar_add(
                                biased[:, ch, :],
                                acc[:, ch].rearrange("p a b t -> p (a b t)"),
                                bc2r_sb[:, ch:ch + 1])
                        am = stp.tile([128, 1], f32, tag="am")
                        nc.vector.tensor_reduce(
                            out=am[:], in_=biased[:], axis=mybir.AxisListType.XY,
                            op=mybir.AluOpType.max, apply_absolute_value=True)
                        nc.vector.tensor_copy(qs_all[:, ds(g, 1)], am[:])
                        sc = stp.tile([128, 1], f32, tag="sc")
                        nc.vector.reciprocal(sc[:], am[:])
                        nc.vector.tensor_scalar_mul(sc[:], sc[:], 127.0)
                        strip = stp.tile([128, 2 * GRP * TPW], i8, tag="strip")
                        nc.vector.tensor_tensor(
                            out=strip[:],
                            in0=biased[:].rearrange("p a b -> p (a b)"),
                            in1=sc[:].to_broadcast([128, 2 * GRP * TPW]),
                            op=MUL)
                        nc.sync.dma_start(
                            out=out2[:, ds(g * (GRP * TPW * 2), GRP * TPW * 2)],
                            in_=strip[:])
                nc.sync.dma_start(out=qs[:], in_=qs_all[:])

    nc.compile()
    return nc


def _get_nc(tpc):
    key = (tpc, PHASES, Q4)
    if key not in _NC_CACHE:
        _NC_CACHE[key] = _build(tpc, PHASES, Q4)
    return _NC_CACHE[key]


def _prep(x, edge_index, w1l, b1l, w1r, w2l, b2l, w2r, wc1, bc1, wc2, bc2):
    x = np.asarray(x, dtype=np.float32)
    ei = np.asarray(edge_index)
    src = ei[0].astype(np.int64)
    dst = ei[1].astype(np.int64)
    e_tot = src.shape[0]

    core_of = (dst // NS).astype(np.int64)
    win_of = ((dst % NS) // W).astype(np.int64)
    dloc = ((dst % NS) % W).astype(np.int8)
    prow_src = (src // NS) * BLK + (src % NS)
    chunk_of = prow_src // CHUNK
    gkey = (core_of * NWIN + win_of) * NCHUNK + chunk_of
    perm = np.argsort(gkey, kind='stable')
    gk_s = gkey[perm]
    counts = np.bincount(gkey, minlength=NCORES * NWIN * NCHUNK)
    tpc = max(5, int(np.ceil(counts.max() / W)))
    TPW = NCHUNK * tpc
    SLOTS = NWIN * TPW * W
    CALL = GRP * tpc * W

    starts = np.zeros(NCORES * NWIN * NCHUNK + 1, np.int64)
    np.cumsum(counts, out=starts[1:])
    pos_in_group = np.arange(e_tot) - starts[gk_s]
    k_p = gk_s // (NWIN * NCHUNK)
    wc_p = gk_s % (NWIN * NCHUNK)
    slot = wc_p * (tpc * W) + pos_in_group

    src16 = np.zeros((NCORES, SLOTS), np.int16)
    dstloc = np.full((NCORES, SLOTS), -1, np.int8)
    orig = np.full((NCORES, SLOTS), -1, np.int64)
    src16[k_p, slot] = (prow_src - chunk_of * CHUNK)[perm].astype(np.int16)
    dstloc[k_p, slot] = dloc[perm]
    orig[k_p, slot] = perm

    def wrap16(a):
        # [..., n] -> [..., 16, n//16]: idx j at (j%16, j//16)
        sh = a.shape[:-1]
        n = a.shape[-1]
        return a.reshape(sh + (n // 16, 16)).swapaxes(-1, -2)

    def band_pack(blocks, dt):
        # blocks [NCORES, NGRP, NCHUNK, CALL]: chunk c -> rows 16c..16c+16
        # (on-device replication fills the second 16-row copy of each band)
        out = np.zeros((NCORES, 64, NGRP * (CALL // 16)), dt)
        wr = wrap16(blocks)                   # [k, g, c, 16, CALL//16]
        for c in range(NCHUNK):
            band = wr[:, :, c].transpose(0, 2, 1, 3).reshape(
                NCORES, 16, NGRP * (CALL // 16))
            out[:, 16 * c:16 * c + 16] = band
        return out

    s5 = src16.reshape(NCORES, NGRP, GRP, NCHUNK, tpc * W)
    s5 = s5.transpose(0, 1, 3, 2, 4).reshape(NCORES, NGRP, NCHUNK, CALL)
    src_w = band_pack(s5, np.int16)

    d5 = dstloc.reshape(NCORES, NWIN, NCHUNK, tpc, W)
    dst_w = np.ascontiguousarray(
        d5.transpose(0, 4, 1, 2, 3).reshape(NCORES, 128, NWIN * TPW))
    iota_cols = np.broadcast_to(np.arange(128, dtype=np.int8), (128, 128))
    dst_w = np.concatenate(
        [dst_w, np.broadcast_to(iota_cols, (NCORES, 128, 128))], axis=2)
    dst_w = np.ascontiguousarray(dst_w)

    # clamp pad slots (-1) to 0: their gathered rows are discarded, but a
    # negative index would make the v gather read out of bounds
    v5 = np.maximum(dstloc, 0).reshape(NCORES, NGRP, GRP, NCHUNK, tpc * W)
    v5 = v5.transpose(0, 1, 3, 2, 4).reshape(NCORES, NGRP, NCHUNK, CALL)
    vdl_w = band_pack(v5, np.int8)

    xc = np.zeros((TROWS, 64), np.float16)
    prow_all = (np.arange(N) // NS) * BLK + (np.arange(N) % NS)
    xc[prow_all] = x.astype(np.float16)

    w1l = np.asarray(w1l, np.float32); w1r = np.asarray(w1r, np.float32)
    w2l = np.asarray(w2l, np.float32); w2r = np.asarray(w2r, np.float32)
    wc1 = np.asarray(wc1, np.float32); wc2 = np.asarray(wc2, np.float32)
    wpack = np.concatenate([
        np.eye(128, dtype=np.float16).ravel(),
        np.ascontiguousarray(w1l.T).astype(np.float16).ravel(),
        np.ascontiguousarray(w1r.T).astype(np.float16).ravel(),
        np.ascontiguousarray(w2l.T).astype(np.float16).ravel(),
        np.ascontiguousarray(w2r.T).astype(np.float16).ravel(),
        np.ascontiguousarray(wc1[:, :64].T).astype(np.float16).ravel(),
        np.ascontiguousarray(wc1[:, 64:].T).astype(np.float16).ravel(),
        np.broadcast_to(wc2.astype(np.float16).reshape(1, 128),
                        (128, 128)).ravel(),
    ])
    bpack = np.concatenate([
        np.asarray(b1l, np.float32).ravel(),
        np.asarray(b2l, np.float32).ravel(),
        np.asarray(bc1, np.float32).ravel(),
        np.broadcast_to(np.asarray(bc2, np.float32), (128, 2)).ravel(),
    ])
    const_u8 = np.concatenate(
        [wpack.view(np.uint8).ravel(), bpack.view(np.uint8).ravel()])

    in_maps = []
    for k in range(NCORES):
        blob = np.concatenate([
            xc[k * BLK:(k + 1) * BLK].view(np.uint8).ravel(),
            src_w[k].view(np.uint8).ravel(),
            vdl_w[k].view(np.uint8).ravel(),
            dst_w[k].view(np.uint8).ravel(),
            const_u8,
        ]).view(np.float16)
        in_maps.append({"blob": blob})

    meta = {"tpc": tpc, "orig": orig, "src16": src16, "dstloc": dstloc,
            "e_tot": e_tot}
    return in_maps, meta


def _unscramble(results, meta):
    # out2 is [128 pos, cols]; cols ordered (group, ch, win-in-grp, chunk,
    # tile)
    tpc = meta["tpc"]; orig = meta["orig"]; e_tot = meta["e_tot"]
    TPW = NCHUNK * tpc
    out = np.zeros((e_tot, 2), np.float32)
    w_arr = np.arange(NWIN)[:, None, None]
    c_arr = np.arange(NCHUNK)[None, :, None]
    t_arr = np.arange(tpc)[None, None, :]
    colbase = ((w_arr // GRP) * (2 * GRP * TPW)
               + (w_arr % GRP) * TPW + c_arr * tpc + t_arr)
    colbase = np.repeat(colbase.reshape(NWIN * TPW), W)
    p_arr = np.tile(np.arange(W), NWIN * TPW)
    g_arr = np.repeat(np.arange(NWIN) // GRP, TPW * W)
    for k in range(NCORES):
        o2 = np.asarray(results[k]["out2"]).astype(np.float32)
        sc = np.asarray(results[k]["qs"]) / 127.0      # [128, NGRP]
        valid = orig[k] >= 0
        deq = sc[p_arr[valid], g_arr[valid]]
        out[orig[k][valid], 0] = o2[p_arr[valid], colbase[valid]] * deq
        out[orig[k][valid], 1] = o2[p_arr[valid], colbase[valid] + GRP * TPW] * deq
    return out


def kernel(**inputs):
    global LAST_EXEC_TIME_NS, LAST_RUN_WALL_NS
    in_maps, meta = _prep(**inputs)
    nc = _get_nc(meta["tpc"])
    import time as _time
    _t0 = _time.time()
    res = run_bass_kernel_spmd(nc, in_maps, list(range(NCORES)), trace=TRACE)
    LAST_RUN_WALL_NS = int((_time.time() - _t0) * 1e9)
    LAST_EXEC_TIME_NS = res.exec_time_ns
    if PHASES < 3:
        return res.results, meta
    return _unscramble(res.results, meta)


# revision 41
# speedup vs baseline: 1.2470x; 1.2470x over previous
"""Trainium2 Bass kernel for the DiseaseGNN problem (2x SAGEConv + edge MLP).

Strategy (8 NeuronCores, SPMD):
  - Edges sorted by dst; core k owns dst range [12500k, 12500(k+1)).
  - Aggregation = one-hot matmuls accumulated in PSUM per 128-node window
    (no scatter needed). Counts ride along as a ones-column in the gathered
    message tile.
  - Per-edge gathers via the dma_gather extended instruction (int16 indices);
    tables are laid out as 8 blocks of 12544 padded rows (100352 total) and
    split into 4 chunks of 25088 rows so local indices fit in int16.
  - h = relu(W_l @ mean + W_r @ h_prev) computed in node space per window.
  - Classifier folded into node space: u = h2@wc1a.T + bc1, v = h2@wc1b.T,
    hidden = relu(u[src] + v[dst]), out = hidden@wc2.T + bc2, with u and v
    packed as one [u|v] per-node table: the AllGathered copy serves u[src],
    the local copy serves v[dst] (dst is always core-local).
  - x is shipped as one 1.6MB block per core and AllGathered + expanded to
    the 256-byte-row gather table on device; its transpose (for the lin_r
    term) is built on device with a DMA transpose.
  - Host<->device traffic minimized: indices shipped un-replicated (the
    2x16-row band replication the gather needs is done on device), all
    weights/biases packed into two params, iota packed into the dst param,
    fp16 output.
"""
import os
import sys
import numpy as np

for _p in ('/opt/trn_rl_repo',):
    if _p not in sys.path:
        sys.path.insert(0, _p)

# Persistent compilation caches: run_bass_kernel_spmd re-jits its closure on
# every call, so without a persistent cache each call pays the full XLA
# compile (~4s). With it, repeat calls hit disk.
try:
    import jax
    _cache_dir = os.path.expanduser("~/.cache/bass_jax_cache")
    os.makedirs(_cache_dir, exist_ok=True)
    jax.config.update("jax_compilation_cache_dir", _cache_dir)
    jax.config.update("jax_persistent_cache_min_entry_size_bytes", 0)
    jax.config.update("jax_persistent_cache_min_compile_time_secs", 0.0)
except Exception:
    pass

import concourse.bass as bass
import concourse.bacc as bacc
import concourse.mybir as mybir
import concourse.tile as tile
from concourse.bass import ds
from concourse.bass_utils import run_bass_kernel_spmd

f32 = mybir.dt.float32
fp16 = mybir.dt.float16
i16 = mybir.dt.int16
i8 = mybir.dt.int8

N = 100000
E = 1600000
NCORES = 8
NS = N // NCORES            # 12500 nodes per core
W = 128                      # node window
NWIN = (NS + W - 1) // W     # 98 windows per core (last partial)
BLK = NWIN * W               # 12544 padded table rows per core block
TROWS = NCORES * BLK         # 100352
NCHUNK = 4
CHUNK = TROWS // NCHUNK      # 25088 (< 32767 so int16 indices work)
GRP = 2                      # windows per gather group
NGRP = NWIN // GRP           # 49

TRACE = False
LAST_EXEC_TIME_NS = None
LAST_RUN_WALL_NS = None
PHASES = 3                   # debug: 1=layer1 only, 2=+layer2, 3=full
Q4 = True                    # debug: rotate gathers across 4 SWDGE queues

_NC_CACHE = {}

RELU = mybir.ActivationFunctionType.Relu
IDENT = mybir.ActivationFunctionType.Identity
EQ = mybir.AluOpType.is_equal
MUL = mybir.AluOpType.mult


def _build(tpc, phases=3, q4=True):
    TPW = NCHUNK * tpc                 # tiles per window
    CALL = GRP * tpc * W               # idx per (group, chunk) gather call
    SLOTS_COLS = NWIN * TPW * 2        # out2 cols
    IDXC = NGRP * (CALL // 16)         # idx columns (i16, wrap-16)

    nc = bacc.Bacc(num_swdge_queues=4)

    # The axon transfer path has ~100ms fixed cost PER BUFFER, so all inputs
    # ship as one packed fp16 blob, unpacked on device via bitcast APs.
    # Section lengths in fp16 halfwords (all even, so f32 views stay aligned):
    WTOT = 128 * 128 + 64 * 128 * 2 + 128 * 64 * 2 + 64 * 64 * 2 + 128 * 128
    L_XC = BLK * 64                    # x block, fp16
    L_SRC = 64 * IDXC                  # src idx bands, i16 (un-replicated)
    L_VDL = 64 * IDXC // 2             # v dst-local bands, i8
    L_DST = 128 * (NWIN * TPW + 128) // 2   # dst one-hot operand + iota, i8
    L_W = WTOT                         # packed fp16 consts: ident w1lT w1rT
    #                                    w2lT w2rT wc1aT wc1bT wc2rep
    L_B = 512 * 2                      # packed f32 biases: b1l b2l bc1 bc2rep
    O_XC = 0
    O_SRC = O_XC + L_XC
    O_VDL = O_SRC + L_SRC
    O_DST = O_VDL + L_VDL
    O_W = O_DST + L_DST
    O_B = O_W + L_W
    TOTH = O_B + L_B
    blob = nc.declare_dram_parameter("blob", [TOTH], fp16, isOutput=False)
    # pos-major int8 output; per-group col blocks are [ch, win-in-grp,
    # chunk, tile], dynamically quantized per (partition, group); the f32
    # scales live in the last NGRP*4 columns (bitcast) so everything ships
    # as one buffer
    out2 = nc.declare_dram_parameter(
        "out2", [128, SLOTS_COLS + NGRP * 4], i8, isOutput=True)
    if phases < 3:
        out_h1 = nc.declare_dram_parameter("out_h1", [128, BLK], fp16, isOutput=True)
        out_u = nc.declare_dram_parameter("out_u", [BLK, 128], fp16, isOutput=True)
        out_v = nc.declare_dram_parameter("out_v", [BLK, 128], fp16, isOutput=True)

    xc_b = nc.dram_tensor("xc_bounce", [BLK, 64], fp16)
    xc_full = nc.dram_tensor("xc_full", [TROWS, 64], fp16, addr_space="Shared")
    xp = nc.dram_tensor("xp_int", [TROWS, 128], fp16)
    h1_local = nc.dram_tensor("h1_local", [BLK, 128], fp16)
    h1_full = nc.dram_tensor("h1_full", [TROWS, 128], fp16, addr_space="Shared")
    u_local = nc.dram_tensor("u_local", [BLK, 128], fp16)
    u_full = nc.dram_tensor("u_full", [TROWS, 128], fp16, addr_space="Shared")

    with tile.TileContext(nc) as tc:
        with (
            tc.tile_pool(name="const", bufs=1) as const,
            tc.tile_pool(name="resident", bufs=1) as res,
        ):
            # --- unpack consts ---
            woff = [O_W]

            def wload(p, cols):
                t = const.tile([p, cols], fp16, tag=f"w{woff[0]}")
                nc.sync.dma_start(
                    out=t[:],
                    in_=blob[woff[0]:woff[0] + p * cols].rearrange(
                        "(p x) -> p x", p=p))
                woff[0] += p * cols
                return t

            ident_sb = wload(128, 128)
            w1lT_sb = wload(64, 128)
            w1rT_sb = wload(64, 128)
            w2lT_sb = wload(128, 64)
            w2rT_sb = wload(128, 64)
            wc1aT_sb = wload(64, 64)
            wc1bT_sb = wload(64, 64)
            wc2r_sb = wload(128, 128)   # wc2 rows replicated: [p, ch*64+f]

            boff = [O_B]

            def bload(p, cols):
                t = const.tile([p, cols], f32, tag=f"b{boff[0]}")
                nc.sync.dma_start(
                    out=t[:],
                    in_=blob[boff[0]:boff[0] + p * cols * 2].bitcast(
                        f32).rearrange("(p x) -> p x", p=p))
                boff[0] += p * cols * 2
                return t

            b1l_sb = bload(128, 1)
            b2l_sb = bload(64, 1)
            bc1_sb = bload(64, 1)
            bc2r_sb = bload(128, 2)

            # resident idx/dst tables, loaded once and reused by all phases
            src_idx = res.tile([128, IDXC], i16)
            vdl8 = res.tile([128, IDXC], i8)
            v_idx = res.tile([128, IDXC], i16)
            dst_all = res.tile([128, NWIN * TPW + 128], i8)
            nc.sync.dma_start(
                out=dst_all[:],
                in_=blob[O_DST:O_DST + L_DST].bitcast(i8).rearrange(
                    "(p x) -> p x", p=128))
            for c in range(NCHUNK):
                for h in range(2):
                    nc.sync.dma_start(
                        out=src_idx[32 * c + 16 * h:32 * c + 16 * (h + 1), :],
                        in_=blob[O_SRC + 16 * c * IDXC:
                                 O_SRC + 16 * (c + 1) * IDXC].bitcast(
                            i16).rearrange("(r x) -> r x", r=16))
                    nc.sync.dma_start(
                        out=vdl8[32 * c + 16 * h:32 * c + 16 * (h + 1), :],
                        in_=blob[O_VDL + 8 * c * IDXC:
                                 O_VDL + 8 * (c + 1) * IDXC].bitcast(
                            i8).rearrange("(r x) -> r x", r=16))

            # x in SBUF, window-major: xcb_t[p, a, d] = xc[a*128+p, d]
            xcb_t = res.tile([128, NWIN, 64], fp16)
            nc.sync.dma_start(
                out=xcb_t[:],
                in_=blob[O_XC:O_XC + L_XC].rearrange(
                    "(a p d) -> p a d", p=128, d=64))
            # x^T built on device with per-window PE transposes (saves
            # shipping a second copy of x)
            xT_sb = res.tile([64, BLK], fp16)

            h1T_all = res.tile([128, BLK], fp16)
            recip_all = res.tile([128, NWIN], f32)
            qs_all = res.tile([128, NGRP], f32)

            HCALL = tpc * W                      # 640 idx per (window, chunk) call
            HC16 = HCALL // 16

            # v idx (dst row in the local u|v table) = dst_local + 128*w,
            # derived on device: iota makes the per-window 128*w offsets
            wofs = res.tile([128, IDXC], i16)
            nc.gpsimd.iota(wofs[:], pattern=[[128, NWIN], [0, HC16]],
                           channel_multiplier=0)
            nc.vector.tensor_tensor(out=v_idx[:], in0=vdl8[:], in1=wofs[:],
                                    op=mybir.AluOpType.add)

            def gather_win(idx_src, msgp, w, table):
                """Per-(window, chunk) gathers: 640 idx = 41 ring descs, under
                the ~64-desc SWDGE carveout ring limit. Queue c's Q7 pair
                reads idx partitions 32c..32c+31. `w` may be symbolic."""
                msgs = []
                for c in range(NCHUNK):
                    m = msgp.tile([128, tpc, 128], fp16, tag=f"msg{c}")
                    nc.gpsimd.dma_gather(
                        out_ap=m[:], in_ap=table[c * CHUNK:(c + 1) * CHUNK, :],
                        idxs_ap=idx_src[:, ds(w * HC16, HC16)],
                        num_idxs=HCALL, num_idxs_reg=HCALL,
                        elem_size=128, queue_num=c)
                    msgs.append(m)
                return msgs

            def onehot_for(ohp, w):
                oh = ohp.tile([128, TPW, 128], fp16)
                nc.vector.tensor_tensor(
                    out=oh[:],
                    in0=dst_all[:, ds(w * TPW, TPW)].rearrange(
                        "p (b o) -> p b o", o=1).to_broadcast([128, TPW, 128]),
                    in1=dst_all[:, NWIN * TPW:].rearrange(
                        "p (o q) -> p o q", o=1).to_broadcast([128, TPW, 128]),
                    op=EQ)
                return oh

            # AllGather each core's x block, then expand to the 256B-row
            # gather table (cols 0:64 = x, col 64 = 1.0) on device; ships
            # 1.6MB instead of 12.8MB of x per core.
            nc.sync.dma_start(
                out=xc_b[:].rearrange("(a p) d -> p a d", p=128),
                in_=xcb_t[:])
            with (
                tc.tile_pool(name="xw", bufs=3) as xwp,
                tc.tile_pool(name="psXT", bufs=4, space="PSUM") as psxt,
            ):
                with tc.For_i(0, NWIN, 1) as w:
                    xw = xwp.tile([128, 64], fp16, tag="xw")
                    nc.vector.tensor_copy(
                        xw[:],
                        xcb_t[:, ds(w, 1), :].rearrange("p a d -> p (a d)"))
                    xtp = psxt.tile([64, 128], fp16, tag="xt")
                    nc.tensor.transpose(xtp[:], xw[:], ident_sb[:])
                    nc.vector.tensor_copy(xT_sb[:, ds(w * 128, 128)], xtp[:])
            tc.strict_bb_all_engine_barrier()
            with tc.tile_critical():
                with nc.semaphore() as cc_sem:
                    nc.gpsimd.collective_compute(
                        "AllGather", mybir.AluOpType.bypass,
                        ins=[xc_b[:]], outs=[xc_full[:]],
                        replica_groups=[list(range(NCORES))],
                    ).then_inc(cc_sem, 1)
                    nc.gpsimd.wait_ge(cc_sem, 1)
            tc.strict_bb_all_engine_barrier()
            with tc.tile_pool(name="expand", bufs=2) as exp_p:
                ones_t = exp_p.tile([128, NWIN, 1], fp16, tag="ones")
                nc.vector.memset(ones_t[:], 1.0)
                for blk_i in range(NCORES):
                    r0 = blk_i * BLK
                    t = exp_p.tile([128, NWIN, 64], fp16, tag="xstripe")
                    nc.sync.dma_start(
                        out=t[:],
                        in_=xc_full[r0:r0 + BLK, :].rearrange("(a p) d -> p a d", p=128))
                    nc.sync.dma_start(
                        out=xp[r0:r0 + BLK, 0:64].rearrange("(a p) d -> p a d", p=128),
                        in_=t[:])
                    nc.sync.dma_start(
                        out=xp[r0:r0 + BLK, 64:65].rearrange("(a p) d -> p a d", p=128),
                        in_=ones_t[:])
            tc.strict_bb_all_engine_barrier()

            # ---------------- Phase A: layer 1 ----------------
            with (
                tc.tile_pool(name="msgA", bufs=3) as msgp,
                tc.tile_pool(name="ohA", bufs=3) as ohp,
                tc.tile_pool(name="smallA", bufs=4) as smp,
                tc.tile_pool(name="psAggA", bufs=2, space="PSUM") as psagg,
                tc.tile_pool(name="psTrA", bufs=3, space="PSUM") as pstr,
                tc.tile_pool(name="psHA", bufs=2, space="PSUM") as psh,
            ):
                with tc.For_i(0, NWIN, 1) as w:
                    msgs = gather_win(src_idx, msgp, w, xp)
                    oh = onehot_for(ohp, w)
                    agg = psagg.tile([128, 65], f32)
                    for c in range(NCHUNK):
                        for t in range(tpc):
                            nc.tensor.matmul(
                                out=agg[:], lhsT=oh[:, c * tpc + t, :],
                                rhs=msgs[c][:, t, :65],
                                start=(c == 0 and t == 0),
                                stop=(c == NCHUNK - 1 and t == tpc - 1))
                    cntm = smp.tile([128, 1], f32, tag="cnt")
                    nc.vector.tensor_scalar_max(cntm[:], agg[:, 64:65], 1.0)
                    recip = smp.tile([128, 1], f32, tag="recip")
                    nc.vector.reciprocal(recip[:], cntm[:])
                    nc.vector.tensor_copy(recip_all[:, ds(w, 1)], recip[:])
                    mean = smp.tile([128, 64], fp16, tag="mean")
                    nc.vector.tensor_tensor(
                        out=mean[:], in0=agg[:, :64],
                        in1=recip[:].to_broadcast([128, 64]), op=MUL)
                    meanT_ps = pstr.tile([64, 128], fp16, tag="tr")
                    nc.tensor.transpose(meanT_ps[:], mean[:], ident_sb[:])
                    meanT = smp.tile([64, 128], fp16, tag="meanTs")
                    nc.vector.tensor_copy(meanT[:], meanT_ps[:])
                    h1ps = psh.tile([128, 128], f32, tag="h1")
                    nc.tensor.matmul(h1ps[:], lhsT=w1lT_sb[:], rhs=meanT[:], start=True, stop=False)
                    nc.tensor.matmul(h1ps[:], lhsT=w1rT_sb[:], rhs=xT_sb[:, ds(w * 128, 128)],
                                     start=False, stop=True)
                    h1T = smp.tile([128, 128], fp16, tag="h1T")
                    nc.scalar.activation(out=h1T[:], in_=h1ps[:],
                                         func=RELU, bias=b1l_sb[:])
                    nc.vector.tensor_copy(h1T_all[:, ds(w * 128, 128)], h1T[:])
                    h1nm_ps = pstr.tile([128, 128], fp16, tag="tr")
                    nc.tensor.transpose(h1nm_ps[:], h1T[:], ident_sb[:])
                    h1nm = smp.tile([128, 128], fp16, tag="h1nms")
                    nc.vector.tensor_copy(h1nm[:], h1nm_ps[:])
                    nc.sync.dma_start(out=h1_local[ds(w * 128, 128), :], in_=h1nm[:])

            if phases < 3:
                nc.sync.dma_start(out=out_h1[:], in_=h1T_all[:])
            if phases >= 2:
                tc.strict_bb_all_engine_barrier()
                with tc.tile_critical():
                    with nc.semaphore() as cc_sem:
                        nc.gpsimd.collective_compute(
                            "AllGather", mybir.AluOpType.bypass,
                            ins=[h1_local[:]], outs=[h1_full[:]],
                            replica_groups=[list(range(NCORES))],
                        ).then_inc(cc_sem, 1)
                        nc.gpsimd.wait_ge(cc_sem, 1)
                tc.strict_bb_all_engine_barrier()

                # ------------- Phase B: layer 2 + u/v tables -------------
                with (
                    tc.tile_pool(name="msgB", bufs=3) as msgp,
                    tc.tile_pool(name="ohB", bufs=3) as ohp,
                    tc.tile_pool(name="smallB", bufs=4) as smp,
                    tc.tile_pool(name="psAggB", bufs=2, space="PSUM") as psagg,
                    tc.tile_pool(name="psTrB", bufs=3, space="PSUM") as pstr,
                    tc.tile_pool(name="psHB", bufs=3, space="PSUM") as psh,
                ):
                    with tc.For_i(0, NWIN, 1) as w:
                        msgs = gather_win(src_idx, msgp, w, h1_full)
                        oh = onehot_for(ohp, w)
                        agg2 = psagg.tile([128, 128], f32)
                        for c in range(NCHUNK):
                            for t in range(tpc):
                                nc.tensor.matmul(
                                    out=agg2[:], lhsT=oh[:, c * tpc + t, :],
                                    rhs=msgs[c][:, t, :],
                                    start=(c == 0 and t == 0),
                                    stop=(c == NCHUNK - 1 and t == tpc - 1))
                        mean2 = smp.tile([128, 128], fp16, tag="mean2")
                        nc.vector.tensor_tensor(
                            out=mean2[:], in0=agg2[:],
                            in1=recip_all[:, ds(w, 1)].to_broadcast([128, 128]), op=MUL)
                        mean2T_ps = pstr.tile([128, 128], fp16, tag="tr")
                        nc.tensor.transpose(mean2T_ps[:], mean2[:], ident_sb[:])
                        mean2T = smp.tile([128, 128], fp16, tag="m2Ts")
                        nc.vector.tensor_copy(mean2T[:], mean2T_ps[:])
                        h2ps = psh.tile([64, 128], f32, tag="h")
                        nc.tensor.matmul(h2ps[:], lhsT=w2lT_sb[:], rhs=mean2T[:], start=True, stop=False)
                        nc.tensor.matmul(h2ps[:], lhsT=w2rT_sb[:], rhs=h1T_all[:, ds(w * 128, 128)],
                                         start=False, stop=True)
                        h2T = smp.tile([64, 128], fp16, tag="h2T")
                        nc.scalar.activation(out=h2T[:], in_=h2ps[:], func=RELU, bias=b2l_sb[:])
                        ups = psh.tile([64, 128], f32, tag="h")
                        nc.tensor.matmul(ups[:], lhsT=wc1aT_sb[:], rhs=h2T[:], start=True, stop=True)
                        uT = smp.tile([64, 128], fp16, tag="uT")
                        nc.scalar.activation(out=uT[:], in_=ups[:], func=IDENT, bias=bc1_sb[:])
                        vps = psh.tile([64, 128], f32, tag="h")
                        nc.tensor.matmul(vps[:], lhsT=wc1bT_sb[:], rhs=h2T[:], start=True, stop=True)
                        vT = smp.tile([64, 128], fp16, tag="vT")
                        nc.vector.tensor_copy(vT[:], vps[:])
                        unm_ps = pstr.tile([128, 64], fp16, tag="tr")
                        nc.tensor.transpose(unm_ps[:], uT[:], ident_sb[0:64, 0:64])
                        unm = smp.tile([128, 64], fp16, tag="unms")
                        nc.vector.tensor_copy(unm[:], unm_ps[:])
                        nc.sync.dma_start(out=u_local[ds(w * 128, 128), 0:64], in_=unm[:])
                        vnm_ps = pstr.tile([128, 64], fp16, tag="tr")
                        nc.tensor.transpose(vnm_ps[:], vT[:], ident_sb[0:64, 0:64])
                        vnm = smp.tile([128, 64], fp16, tag="vnms")
                        nc.vector.tensor_copy(vnm[:], vnm_ps[:])
                        nc.sync.dma_start(out=u_local[ds(w * 128, 128), 64:128], in_=vnm[:])

            if phases >= 3:
                tc.strict_bb_all_engine_barrier()
                with tc.tile_critical():
                    with nc.semaphore() as cc_sem:
                        nc.gpsimd.collective_compute(
                            "AllGather", mybir.AluOpType.bypass,
                            ins=[u_local[:]], outs=[u_full[:]],
                            replica_groups=[list(range(NCORES))],
                        ).then_inc(cc_sem, 1)
                        nc.gpsimd.wait_ge(cc_sem, 1)
                tc.strict_bb_all_engine_barrier()

                # ------------------ Phase C: classifier ------------------
                # hidden = relu(u[src] + v[dst]); out[e,ch] = hidden.wc2[ch]
                # + bc2[ch], all on DVE/ACT: fused add/relu per chunk-call,
                # then per-channel multiply + innermost-dim reduce.
                ADD = mybir.AluOpType.add
                with (
                    tc.tile_pool(name="gC", bufs=3) as gp,
                    tc.tile_pool(name="hC", bufs=3) as hp,
                    tc.tile_pool(name="accC", bufs=2) as accp,
                    tc.tile_pool(name="stripC", bufs=3) as stp,
                ):
                    with tc.For_i(0, NGRP, 1) as g:
                        acc = accp.tile([128, 2, GRP, NCHUNK, tpc], f32)
                        for wi in range(GRP):
                            for c in range(NCHUNK):
                                ub = gp.tile([128, tpc, 128], fp16, tag=f"ubuf{c}")
                                nc.gpsimd.dma_gather(
                                    out_ap=ub[:], in_ap=u_full[c * CHUNK:(c + 1) * CHUNK, :],
                                    idxs_ap=src_idx[:, ds((g * GRP + wi) * HC16, HC16)],
                                    num_idxs=HCALL, num_idxs_reg=HCALL,
                                    elem_size=128, queue_num=c)
                                vb = gp.tile([128, tpc, 128], fp16, tag=f"vbuf{c}")
                                nc.gpsimd.dma_gather(
                                    out_ap=vb[:], in_ap=u_local[:],
                                    idxs_ap=v_idx[:, ds((g * GRP + wi) * HC16, HC16)],
                                    num_idxs=HCALL, num_idxs_reg=HCALL,
                                    elem_size=128, queue_num=c)
                                hs = hp.tile([128, tpc, 64], fp16, tag="hs")
                                nc.vector.tensor_add(
                                    hs[:], ub[:, :, 0:64], vb[:, :, 64:128])
                                hid = hp.tile([128, tpc, 64], fp16, tag="hid")
                                nc.scalar.activation(out=hid[:], in_=hs[:], func=RELU)
                                for ch in range(2):
                                    prod = hp.tile([128, tpc, 64], fp16, tag="prod")
                                    nc.vector.tensor_tensor(
                                        out=prod[:], in0=hid[:],
                                        in1=wc2r_sb[:, ch * 64:(ch + 1) * 64].rearrange(
                                            "p (o f) -> p o f", o=1).to_broadcast(
                                            [128, tpc, 64]),
                                        op=MUL)
                                    nc.vector.tensor_reduce(
                                        out=acc[:, ch, wi, c, :], in_=prod[:],
                                        axis=mybir.AxisListType.X, op=ADD)
                        biased = stp.tile([128, 2, GRP * TPW], f32, tag="biased")
                        for ch in range(2):
                            nc.vector.tensor_scalar_add(
                                biased[:, ch, :],
                                acc[:, ch].rearrange("p a b t -> p (a b t)"),
                                bc2r_sb[:, ch:ch + 1])
                        am = stp.tile([128, 1], f32, tag="am")
                        nc.vector.tensor_reduce(
                            out=am[:], in_=biased[:], axis=mybir.AxisListType.XY,
                            op=mybir.AluOpType.max, apply_absolute_value=True)
                        nc.vector.tensor_copy(qs_all[:, ds(g, 1)], am[:])
                        sc = stp.tile([128, 1], f32, tag="sc")
                        nc.vector.reciprocal(sc[:], am[:])
                        nc.vector.tensor_scalar_mul(sc[:], sc[:], 127.0)
                        strip = stp.tile([128, 2 * GRP * TPW], i8, tag="strip")
                        nc.vector.tensor_tensor(
                            out=strip[:],
                            in0=biased[:].rearrange("p a b -> p (a b)"),
                            in1=sc[:].to_broadcast([128, 2 * GRP * TPW]),
                            op=MUL)
                        nc.sync.dma_start(
                            out=out2[:, ds(g * (GRP * TPW * 2), GRP * TPW * 2)],
                            in_=strip[:])
                nc.sync.dma_start(
                    out=out2[:, SLOTS_COLS:].bitcast(f32), in_=qs_all[:])

    nc.compile()
    return nc


def _get_nc(tpc):
    key = (tpc, PHASES, Q4)
    if key not in _NC_CACHE:
        _NC_CACHE[key] = _build(tpc, PHASES, Q4)
    return _NC_CACHE[key]


def _prep(x, edge_index, w1l, b1l, w1r, w2l, b2l, w2r, wc1, bc1, wc2, bc2):
    x = np.asarray(x, dtype=np.float32)
    ei = np.asarray(edge_index)
    src = ei[0].astype(np.int64)
    dst = ei[1].astype(np.int64)
    e_tot = src.shape[0]

    core_of = (dst // NS).astype(np.int64)
    win_of = ((dst % NS) // W).astype(np.int64)
    dloc = ((dst % NS) % W).astype(np.int8)
    prow_src = (src // NS) * BLK + (src % NS)
    chunk_of = prow_src // CHUNK
    gkey = (core_of * NWIN + win_of) * NCHUNK + chunk_of
    perm = np.argsort(gkey, kind='stable')
    gk_s = gkey[perm]
    counts = np.bincount(gkey, minlength=NCORES * NWIN * NCHUNK)
    tpc = max(5, int(np.ceil(counts.max() / W)))
    TPW = NCHUNK * tpc
    SLOTS = NWIN * TPW * W
    CALL = GRP * tpc * W

    starts = np.zeros(NCORES * NWIN * NCHUNK + 1, np.int64)
    np.cumsum(counts, out=starts[1:])
    pos_in_group = np.arange(e_tot) - starts[gk_s]
    k_p = gk_s // (NWIN * NCHUNK)
    wc_p = gk_s % (NWIN * NCHUNK)
    slot = wc_p * (tpc * W) + pos_in_group

    src16 = np.zeros((NCORES, SLOTS), np.int16)
    dstloc = np.full((NCORES, SLOTS), -1, np.int8)
    orig = np.full((NCORES, SLOTS), -1, np.int64)
    src16[k_p, slot] = (prow_src - chunk_of * CHUNK)[perm].astype(np.int16)
    dstloc[k_p, slot] = dloc[perm]
    orig[k_p, slot] = perm

    def wrap16(a):
        # [..., n] -> [..., 16, n//16]: idx j at (j%16, j//16)
        sh = a.shape[:-1]
        n = a.shape[-1]
        return a.reshape(sh + (n // 16, 16)).swapaxes(-1, -2)

    def band_pack(blocks, dt):
        # blocks [NCORES, NGRP, NCHUNK, CALL]: chunk c -> rows 16c..16c+16
        # (on-device replication fills the second 16-row copy of each band)
        out = np.zeros((NCORES, 64, NGRP * (CALL // 16)), dt)
        wr = wrap16(blocks)                   # [k, g, c, 16, CALL//16]
        for c in range(NCHUNK):
            band = wr[:, :, c].transpose(0, 2, 1, 3).reshape(
                NCORES, 16, NGRP * (CALL // 16))
            out[:, 16 * c:16 * c + 16] = band
        return out

    s5 = src16.reshape(NCORES, NGRP, GRP, NCHUNK, tpc * W)
    s5 = s5.transpose(0, 1, 3, 2, 4).reshape(NCORES, NGRP, NCHUNK, CALL)
    src_w = band_pack(s5, np.int16)

    d5 = dstloc.reshape(NCORES, NWIN, NCHUNK, tpc, W)
    dst_w = np.ascontiguousarray(
        d5.transpose(0, 4, 1, 2, 3).reshape(NCORES, 128, NWIN * TPW))
    iota_cols = np.broadcast_to(np.arange(128, dtype=np.int8), (128, 128))
    dst_w = np.concatenate(
        [dst_w, np.broadcast_to(iota_cols, (NCORES, 128, 128))], axis=2)
    dst_w = np.ascontiguousarray(dst_w)

    # clamp pad slots (-1) to 0: their gathered rows are discarded, but a
    # negative index would make the v gather read out of bounds
    v5 = np.maximum(dstloc, 0).reshape(NCORES, NGRP, GRP, NCHUNK, tpc * W)
    v5 = v5.transpose(0, 1, 3, 2, 4).reshape(NCORES, NGRP, NCHUNK, CALL)
    vdl_w = band_pack(v5, np.int8)

    xc = np.zeros((TROWS, 64), np.float16)
    prow_all = (np.arange(N) // NS) * BLK + (np.arange(N) % NS)
    xc[prow_all] = x.astype(np.float16)

    w1l = np.asarray(w1l, np.float32); w1r = np.asarray(w1r, np.float32)
    w2l = np.asarray(w2l, np.float32); w2r = np.asarray(w2r, np.float32)
    wc1 = np.asarray(wc1, np.float32); wc2 = np.asarray(wc2, np.float32)
    wpack = np.concatenate([
        np.eye(128, dtype=np.float16).ravel(),
        np.ascontiguousarray(w1l.T).astype(np.float16).ravel(),
        np.ascontiguousarray(w1r.T).astype(np.float16).ravel(),
        np.ascontiguousarray(w2l.T).astype(np.float16).ravel(),
        np.ascontiguousarray(w2r.T).astype(np.float16).ravel(),
        np.ascontiguousarray(wc1[:, :64].T).astype(np.float16).ravel(),
        np.ascontiguousarray(wc1[:, 64:].T).astype(np.float16).ravel(),
        np.broadcast_to(wc2.astype(np.float16).reshape(1, 128),
                        (128, 128)).ravel(),
    ])
    bpack = np.concatenate([
        np.asarray(b1l, np.float32).ravel(),
        np.asarray(b2l, np.float32).ravel(),
        np.asarray(bc1, np.float32).ravel(),
        np.broadcast_to(np.asarray(bc2, np.float32), (128, 2)).ravel(),
    ])
    const_u8 = np.concatenate(
        [wpack.view(np.uint8).ravel(), bpack.view(np.uint8).ravel()])

    in_maps = []
    for k in range(NCORES):
        blob = np.concatenate([
            xc[k * BLK:(k + 1) * BLK].view(np.uint8).ravel(),
            src_w[k].view(np.uint8).ravel(),
            vdl_w[k].view(np.uint8).ravel(),
            dst_w[k].view(np.uint8).ravel(),
            const_u8,
        ]).view(np.float16)
        in_maps.append({"blob": blob})

    meta = {"tpc": tpc, "orig": orig, "src16": src16, "dstloc": dstloc,
            "e_tot": e_tot}
    return in_maps, meta


def _unscramble(results, meta):
    # out2 is [128 pos, cols]; cols ordered (group, ch, win-in-grp, chunk,
    # tile)
    tpc = meta["tpc"]; orig = meta["orig"]; e_tot = meta["e_tot"]
    TPW = NCHUNK * tpc
    out = np.zeros((e_tot, 2), np.float32)
    w_arr = np.arange(NWIN)[:, None, None]
    c_arr = np.arange(NCHUNK)[None, :, None]
    t_arr = np.arange(tpc)[None, None, :]
    colbase = ((w_arr // GRP) * (2 * GRP * TPW)
               + (w_arr % GRP) * TPW + c_arr * tpc + t_arr)
    colbase = np.repeat(colbase.reshape(NWIN * TPW), W)
    p_arr = np.tile(np.arange(W), NWIN * TPW)
    g_arr = np.repeat(np.arange(NWIN) // GRP, TPW * W)
    SLOTS_COLS = NWIN * TPW * 2
    for k in range(NCORES):
        raw = np.asarray(results[k]["out2"])
        o2 = raw[:, :SLOTS_COLS].astype(np.float32)
        sc = np.ascontiguousarray(raw[:, SLOTS_COLS:]).view(
            np.float32) / 127.0                         # [128, NGRP]
        valid = orig[k] >= 0
        deq = sc[p_arr[valid], g_arr[valid]]
        out[orig[k][valid], 0] = o2[p_arr[valid], colbase[valid]] * deq
        out[orig[k][valid], 1] = o2[p_arr[valid], colbase[valid] + GRP * TPW] * deq
    return out


def kernel(**inputs):
    global LAST_EXEC_TIME_NS, LAST_RUN_WALL_NS
    in_maps, meta = _prep(**inputs)
    nc = _get_nc(meta["tpc"])
    import time as _time
    _t0 = _time.time()
    res = run_bass_kernel_spmd(nc, in_maps, list(range(NCORES)), trace=TRACE)
    LAST_RUN_WALL_NS = int((_time.time() - _t0) * 1e9)
    LAST_EXEC_TIME_NS = res.exec_time_ns
    if PHASES < 3:
        return res.results, meta
    return _unscramble(res.results, meta)
